# revision 22
# baseline (speedup 1.0000x reference)
"""Discriminative-loss (clustering) kernel for Trainium2, 8 NeuronCores.

Strategy: pure data parallelism over the batch (B=16 -> 2 images/core).
Per image, the heavy work is a segmented (per-label) reduction over
524288 pixels:
    sums[l, e]  = sum_p  mask_l(p) * binary(p) * pred[e, p]
    sumsq[l, e] = sum_p  mask_l(p) * binary(p) * pred[e, p]^2
    counts[l]   = sum_p  mask_l(p)
computed on-device; the tiny remaining math (means, hinge terms,
pairwise distances -> scalar loss) is done on the host in float64.

Device mapping per image (pixels viewed as [128 partitions, 4096 cols]):
  - DMA: pred/binary are cast fp32->bf16 on the way in (SWDGE). Random
    rounding errors cancel in the ~1e5-element sums, so the loss keeps
    ~1e-4 accuracy while halving SBUF and letting the PE run at full
    bf16 rate with no moving-size constraints.
  - VectorE: keys t = inst+1 and s = t*binary in bf16, then per label
    bmask_l = [s==l+1] (the matmul weights) and [t==l+1] whose per-
    partition reduction (accum_out) yields the raw pixel counts; the
    is_equal ops run at the 4x DVE mode.
  - ScalarE: pred^2 for all 8 channels in one Square activation over a
    [128, 8, F] view.
  - TensorE: block-diagonal grouped matmul. Each matmul contracts 128
    pixels/column for G=24 columns at once:
      lhsT = bmasks [128, (5 labels, G)]  bf16  -> M = 120 (one
             contiguous group-interleaved block; walrus wants a single
             free dim on the weights AP)
      rhs  = data   [128, (17 chans, G)]  bf16  -> N = 408
    accumulated over all 171 groups of an image into one [120, 408] PSUM
    region. Useful entries are psum[lab*G+j, chan*G+j].
"""

import numpy as np

import concourse.mybir as mybir
from concourse import bacc, bass_utils
from concourse.tile import TileContext

P = 128            # SBUF partitions
F = 480            # pixel columns per pipeline tile (multiple of G); smallish
                   # tiles keep TensorE bursts dense (HAM stays warm) and
                   # deepen the DMA/compute pipeline
G = 24             # pixel-chunk columns per matmul group
NLAB = 5
NCH = 16           # rhs slots: 0-7 pred, 8-15 pred^2
M = NLAB * G       # 120 psum partitions
N = NCH * G        # 408 psum columns
BPC = 2            # images per core
NCORES = 8
DELTA_V = 0.5
DELTA_D = 3.0

# (col0, real_cols, padded_cols) per pipeline tile; 8*480 + 256 = 4096.
TILES = [(i * F, F, F) for i in range(8)] + [(8 * F, 256, 264)]
NT = len(TILES)
NGROUPS = sum(cp // G for _, _, cp in TILES)  # 171

LAST_EXEC_TIME_NS = None

_nc_cache = []


def _build():
    f32, bf16, i32 = mybir.dt.float32, mybir.dt.bfloat16, mybir.dt.int32
    op = mybir.AluOpType

    nc = bacc.Bacc("TRN2", target_bir_lowering=False, num_swdge_queues=4)
    pred = nc.dram_tensor("pred", [BPC, 8, 512, 1024], f32, kind="ExternalInput")
    binl = nc.dram_tensor("binl", [BPC, 512, 1024], f32, kind="ExternalInput")
    inst = nc.dram_tensor("inst", [BPC, 512, 1024], i32, kind="ExternalInput")
    out = nc.dram_tensor("out", [BPC, M, N], f32, kind="ExternalOutput")
    cnt = nc.dram_tensor("cnt", [BPC, P, NT * NLAB], f32, kind="ExternalOutput")

    pred_v = pred.rearrange("b e (p a) w -> b p e (a w)", p=P)  # [2,128,8,4096]
    bin_v = binl.rearrange("b (p a) w -> b p (a w)", p=P)       # [2,128,4096]
    inst_v = inst.rearrange("b (p a) w -> b p (a w)", p=P)

    with TileContext(nc) as tc:
        with tc.tile_pool(name="io", bufs=6) as io, \
             tc.tile_pool(name="wk", bufs=4) as wk, \
             tc.tile_pool(name="ps", bufs=2, space="PSUM") as ps, \
             tc.tile_pool(name="res", bufs=2) as res:
            for b in range(BPC):
                psum = ps.tile([M, N], f32, tag="psum")
                ct = res.tile([P, NT * NLAB], f32, tag="ct")
                k = 0
                for ti, (c0, creal, cpad) in enumerate(TILES):
                    rhs = io.tile([P, NCH, F], bf16, tag="rhs")
                    msk = wk.tile([P, F // G, NLAB * G], bf16, tag="msk")
                    it = io.tile([P, F], i32, tag="it")
                    btf = io.tile([P, F], f32, tag="btf")
                    sk = wk.tile([P, F], bf16, tag="sk")
                    tk = wk.tile([P, F], bf16, tag="tk")
                    junk = wk.tile([P, F], bf16, tag="junk")

                    # pred: fp32 -> bf16 cast inside the SWDGE DMA (Pool
                    # engine); inst/binary ride the cheaper HWDGE path and
                    # binary is cast on DVE.
                    nc.gpsimd.dma_start(out=rhs[:, 0:8, 0:creal],
                                        in_=pred_v[b, :, :, c0:c0 + creal])
                    nc.sync.dma_start(out=btf[:, 0:creal],
                                      in_=bin_v[b, :, c0:c0 + creal])
                    nc.sync.dma_start(out=it[:, 0:creal],
                                      in_=inst_v[b, :, c0:c0 + creal])
                    if cpad > creal:
                        # inst=-1 -> both keys miss every label; pred=0 keeps
                        # NaN garbage out of the accumulated products.
                        nc.vector.memset(it[:, creal:cpad], -1)
                        nc.vector.memset(btf[:, creal:cpad], 0.0)
                        nc.vector.memset(rhs[:, 0:8, creal:cpad], 0.0)

                    nc.vector.tensor_scalar(out=tk[:, 0:cpad], in0=it[:, 0:cpad],
                                            scalar1=1.0, scalar2=None, op0=op.add)
                    nc.vector.tensor_mul(out=sk[:, 0:cpad], in0=tk[:, 0:cpad],
                                         in1=btf[:, 0:cpad])
                    gmax = cpad // G
                    sk_v = sk[:, 0:cpad].rearrange("p (g j) -> p g j", j=G)
                    for lab in range(NLAB):
                        nc.vector.tensor_scalar(out=msk[:, 0:gmax,
                                                        lab * G:(lab + 1) * G],
                                                in0=sk_v,
                                                scalar1=float(lab + 1),
                                                scalar2=None, op0=op.is_equal)
                        # Raw-label compare: tensor discarded, per-partition
                        # sum (accum_out) is the label's pixel count.
                        nc.vector.tensor_scalar(out=junk[:, 0:cpad],
                                                in0=tk[:, 0:cpad],
                                                scalar1=float(lab + 1),
                                                scalar2=0.0, op0=op.is_equal,
                                                op1=op.add,
                                                accum_out=ct[:, ti * NLAB + lab:
                                                             ti * NLAB + lab + 1])
                    nc.scalar.activation(out=rhs[:, 8:16, 0:cpad],
                                         in_=rhs[:, 0:8, 0:cpad],
                                         func=mybir.ActivationFunctionType.Square)

                    for g in range(cpad // G):
                        j0 = g * G
                        nc.tensor.matmul(
                            psum[:, :],
                            msk[:, g, :],
                            rhs[:, :, j0:j0 + G],
                            start=(k == 0),
                            stop=(k == NGROUPS - 1),
                        )
                        k += 1
                ot = res.tile([M, N], f32, tag="ot")
                nc.vector.tensor_copy(out=ot[:, :], in_=psum[:, :])
                nc.gpsimd.dma_start(out=out[b], in_=ot[:, :])
                nc.gpsimd.dma_start(out=cnt[b], in_=ct[:, :])
    # bacc lowering: splits multi-wait sync_info into EventSemaphore
    # instructions (TRN2 allows 1 wait/instruction), inserts ACT table
    # loads, allocates registers.
    nc.compile()
    return nc


def _get_nc():
    if not _nc_cache:
        _nc_cache.append(_build())
    return _nc_cache[0]


def _loss_from_stats(sums, sumsq, counts):
    """Mirror of the reference loss math, in float64. Inputs are [B,5,8],
    [B,5], [B,5]."""
    C = NLAB - 1
    with np.errstate(divide="ignore", invalid="ignore"):
        mu = sums / counts[..., None]                         # [B,5,8]
    frob = sumsq - counts * np.sum(mu * mu, axis=-1)          # [B,5]
    pos = frob > 0
    n = np.where(pos, np.sqrt(np.where(pos, frob, 1.0)), 0.0)
    var = np.where(n > DELTA_V, (n - DELTA_V) ** 2, 0.0)
    l_var = np.sum(var, axis=1) / C                           # [B]

    mu_d = mu[:, :C]                                          # [B,4,8]
    diff = mu_d[:, :, None, :] - mu_d[:, None, :, :]
    dsq = np.sum(diff * diff, axis=-1)                        # [B,4,4]
    offdiag = (1.0 - np.eye(C))[None]
    ok = (dsq > 0) & (offdiag > 0)
    d = np.sqrt(np.where(ok, dsq, 1.0))
    hinge = np.where(ok, np.maximum(DELTA_D - d, 0.0) ** 2,
                     np.where(offdiag > 0, DELTA_D ** 2, 0.0))
    l_dist = np.sum(hinge, axis=(1, 2))                       # [B]
    return np.mean(l_var) + np.mean(l_dist)


def kernel(pred, binary_label, instance_label):
    global LAST_EXEC_TIME_NS
    pred = np.ascontiguousarray(pred, dtype=np.float32)
    binl = np.ascontiguousarray(binary_label, dtype=np.float32).reshape(
        pred.shape[0], 512, 1024)
    inst = np.ascontiguousarray(instance_label, dtype=np.int32)

    nc = _get_nc()
    in_maps = []
    for c in range(NCORES):
        sl = slice(BPC * c, BPC * (c + 1))
        in_maps.append({
            "pred": np.ascontiguousarray(pred[sl]),
            "binl": np.ascontiguousarray(binl[sl]),
            "inst": np.ascontiguousarray(inst[sl]),
        })

    r = bass_utils.run_bass_kernel_spmd(nc, in_maps,
                                        core_ids=list(range(NCORES)))
    LAST_EXEC_TIME_NS = r.exec_time_ns

    S = np.stack([m["out"] for m in r.results]).reshape(
        NCORES * BPC, NLAB, G, NCH, G)
    Sd = np.einsum('bljcj->blc', S).astype(np.float64)        # [16,5,17]
    sums = Sd[:, :, 0:8]
    sumsq = Sd[:, :, 8:16].sum(-1)
    CT = np.stack([m["cnt"] for m in r.results]).reshape(
        NCORES * BPC, P, NT, NLAB)
    counts = CT.astype(np.float64).sum(axis=(1, 2))           # [16,5]

    loss = _loss_from_stats(sums, sumsq, counts)
    return np.array(loss, dtype=np.float32)


# revision 25
# speedup vs baseline: 74044.5221x; 74044.5221x over previous
"""Discriminative-loss (clustering) kernel for Trainium2, 8 NeuronCores.

Strategy: pure data parallelism over the batch (B=16 -> 2 images/core).
Per image, the heavy work is a segmented (per-label) reduction over
524288 pixels:
    sums[l, e]  = sum_p  mask_l(p) * binary(p) * pred[e, p]
    sumsq[l, e] = sum_p  mask_l(p) * binary(p) * pred[e, p]^2
    counts[l]   = sum_p  mask_l(p)
computed on-device; the tiny remaining math (means, hinge terms,
pairwise distances -> scalar loss) is done on the host in float64.

Device mapping per image (pixels viewed as [128 partitions, 4096 cols]):
  - DMA: pred/binary are cast fp32->bf16 on the way in (SWDGE). Random
    rounding errors cancel in the ~1e5-element sums, so the loss keeps
    ~1e-4 accuracy while halving SBUF and letting the PE run at full
    bf16 rate with no moving-size constraints.
  - VectorE: keys t = inst+1 and s = t*binary in bf16, then per label
    bmask_l = [s==l+1] (the matmul weights) and [t==l+1] whose per-
    partition reduction (accum_out) yields the raw pixel counts; the
    is_equal ops run at the 4x DVE mode.
  - ScalarE: pred^2 for all 8 channels in one Square activation over a
    [128, 8, F] view.
  - TensorE: block-diagonal grouped matmul. Each matmul contracts 128
    pixels/column for G=24 columns at once:
      lhsT = bmasks [128, (5 labels, G)]  bf16  -> M = 120 (one
             contiguous group-interleaved block; walrus wants a single
             free dim on the weights AP)
      rhs  = data   [128, (16 chans, G)]  bf16  -> N = 384
    accumulated over all 171 groups of an image into one [120, 384] PSUM
    region. Useful entries are psum[lab*G+j, chan*G+j].
"""

import numpy as np

import concourse.mybir as mybir
from concourse import bacc, bass_utils
from concourse.tile import TileContext

P = 128            # SBUF partitions
F = 432            # pixel columns per pipeline tile (multiple of G); smallish
                   # tiles keep TensorE bursts dense (HAM stays warm) and
                   # deepen the DMA/compute pipeline
G = 24             # pixel-chunk columns per matmul group
NLAB = 5
NCH = 16           # rhs slots: 0-7 pred, 8-15 pred^2
M = NLAB * G       # 120 psum partitions
N = NCH * G        # 408 psum columns
BPC = 2            # images per core
NCORES = 8
DELTA_V = 0.5
DELTA_D = 3.0

# (col0, real_cols, padded_cols) per pipeline tile; 9*432 + 208 = 4096.
TILES = [(i * F, F, F) for i in range(9)] + [(9 * F, 208, 216)]
NT = len(TILES)
NGROUPS = sum(cp // G for _, _, cp in TILES)  # 171

LAST_EXEC_TIME_NS = None

_nc_cache = []


def _build():
    f32, bf16, i32 = mybir.dt.float32, mybir.dt.bfloat16, mybir.dt.int32
    op = mybir.AluOpType

    nc = bacc.Bacc("TRN2", target_bir_lowering=False, num_swdge_queues=4)
    pred = nc.dram_tensor("pred", [BPC, 8, 512, 1024], f32, kind="ExternalInput")
    binl = nc.dram_tensor("binl", [BPC, 512, 1024], f32, kind="ExternalInput")
    inst = nc.dram_tensor("inst", [BPC, 512, 1024], i32, kind="ExternalInput")
    out = nc.dram_tensor("out", [BPC, M, N], f32, kind="ExternalOutput")
    cnt = nc.dram_tensor("cnt", [BPC, P, NT * NLAB], f32, kind="ExternalOutput")

    pred_v = pred.rearrange("b e (p a) w -> b p e (a w)", p=P)  # [2,128,8,4096]
    bin_v = binl.rearrange("b (p a) w -> b p (a w)", p=P)       # [2,128,4096]
    inst_v = inst.rearrange("b (p a) w -> b p (a w)", p=P)

    with TileContext(nc) as tc:
        with tc.tile_pool(name="io", bufs=6) as io, \
             tc.tile_pool(name="wk", bufs=4) as wk, \
             tc.tile_pool(name="ps", bufs=2, space="PSUM") as ps, \
             tc.tile_pool(name="res", bufs=2) as res:
            for b in range(BPC):
                psum = ps.tile([M, N], f32, tag="psum")
                ct = res.tile([P, NT * NLAB], f32, tag="ct")
                k = 0
                for ti, (c0, creal, cpad) in enumerate(TILES):
                    rhs = io.tile([P, NCH, F], bf16, tag="rhs")
                    msk = wk.tile([P, F // G, NLAB * G], bf16, tag="msk")
                    it = io.tile([P, F], i32, tag="it")
                    btf = io.tile([P, F], f32, tag="btf")
                    sk = wk.tile([P, F], bf16, tag="sk")
                    tk = wk.tile([P, F], bf16, tag="tk")
                    junk = wk.tile([P, F], bf16, tag="junk")

                    # pred: fp32 -> bf16 cast inside the SWDGE DMA (Pool
                    # engine); inst/binary ride the cheaper HWDGE path and
                    # binary is cast on DVE.
                    nc.gpsimd.dma_start(out=rhs[:, 0:8, 0:creal],
                                        in_=pred_v[b, :, :, c0:c0 + creal])
                    nc.sync.dma_start(out=btf[:, 0:creal],
                                      in_=bin_v[b, :, c0:c0 + creal])
                    nc.sync.dma_start(out=it[:, 0:creal],
                                      in_=inst_v[b, :, c0:c0 + creal])
                    if cpad > creal:
                        # inst=-1 -> both keys miss every label; pred=0 keeps
                        # NaN garbage out of the accumulated products.
                        nc.vector.memset(it[:, creal:cpad], -1)
                        nc.vector.memset(btf[:, creal:cpad], 0.0)
                        nc.vector.memset(rhs[:, 0:8, creal:cpad], 0.0)

                    nc.vector.tensor_scalar(out=tk[:, 0:cpad], in0=it[:, 0:cpad],
                                            scalar1=1.0, scalar2=None, op0=op.add)
                    nc.vector.tensor_mul(out=sk[:, 0:cpad], in0=tk[:, 0:cpad],
                                         in1=btf[:, 0:cpad])
                    gmax = cpad // G
                    sk_v = sk[:, 0:cpad].rearrange("p (g j) -> p g j", j=G)
                    for lab in range(NLAB):
                        nc.vector.tensor_scalar(out=msk[:, 0:gmax,
                                                        lab * G:(lab + 1) * G],
                                                in0=sk_v,
                                                scalar1=float(lab + 1),
                                                scalar2=None, op0=op.is_equal)
                        # Raw-label compare: tensor discarded, per-partition
                        # sum (accum_out) is the label's pixel count.
                        nc.vector.tensor_scalar(out=junk[:, 0:cpad],
                                                in0=tk[:, 0:cpad],
                                                scalar1=float(lab + 1),
                                                scalar2=0.0, op0=op.is_equal,
                                                op1=op.add,
                                                accum_out=ct[:, ti * NLAB + lab:
                                                             ti * NLAB + lab + 1])
                    nc.scalar.activation(out=rhs[:, 8:16, 0:cpad],
                                         in_=rhs[:, 0:8, 0:cpad],
                                         func=mybir.ActivationFunctionType.Square)

                    for g in range(cpad // G):
                        j0 = g * G
                        nc.tensor.matmul(
                            psum[:, :],
                            msk[:, g, :],
                            rhs[:, :, j0:j0 + G],
                            start=(k == 0),
                            stop=(k == NGROUPS - 1),
                        )
                        k += 1
                ot = res.tile([M, N], f32, tag="ot")
                nc.vector.tensor_copy(out=ot[:, :], in_=psum[:, :])
                nc.gpsimd.dma_start(out=out[b], in_=ot[:, :])
                nc.gpsimd.dma_start(out=cnt[b], in_=ct[:, :])
    # bacc lowering: splits multi-wait sync_info into EventSemaphore
    # instructions (TRN2 allows 1 wait/instruction), inserts ACT table
    # loads, allocates registers.
    nc.compile()
    return nc


def _get_nc():
    if not _nc_cache:
        _nc_cache.append(_build())
    return _nc_cache[0]


def _loss_from_stats(sums, sumsq, counts):
    """Mirror of the reference loss math, in float64. Inputs are [B,5,8],
    [B,5], [B,5]."""
    C = NLAB - 1
    with np.errstate(divide="ignore", invalid="ignore"):
        mu = sums / counts[..., None]                         # [B,5,8]
    frob = sumsq - counts * np.sum(mu * mu, axis=-1)          # [B,5]
    pos = frob > 0
    n = np.where(pos, np.sqrt(np.where(pos, frob, 1.0)), 0.0)
    var = np.where(n > DELTA_V, (n - DELTA_V) ** 2, 0.0)
    l_var = np.sum(var, axis=1) / C                           # [B]

    mu_d = mu[:, :C]                                          # [B,4,8]
    diff = mu_d[:, :, None, :] - mu_d[:, None, :, :]
    dsq = np.sum(diff * diff, axis=-1)                        # [B,4,4]
    offdiag = (1.0 - np.eye(C))[None]
    ok = (dsq > 0) & (offdiag > 0)
    d = np.sqrt(np.where(ok, dsq, 1.0))
    hinge = np.where(ok, np.maximum(DELTA_D - d, 0.0) ** 2,
                     np.where(offdiag > 0, DELTA_D ** 2, 0.0))
    l_dist = np.sum(hinge, axis=(1, 2))                       # [B]
    return np.mean(l_var) + np.mean(l_dist)


def kernel(pred, binary_label, instance_label):
    global LAST_EXEC_TIME_NS
    pred = np.ascontiguousarray(pred, dtype=np.float32)
    binl = np.ascontiguousarray(binary_label, dtype=np.float32).reshape(
        pred.shape[0], 512, 1024)
    inst = np.ascontiguousarray(instance_label, dtype=np.int32)

    nc = _get_nc()
    in_maps = []
    for c in range(NCORES):
        sl = slice(BPC * c, BPC * (c + 1))
        in_maps.append({
            "pred": np.ascontiguousarray(pred[sl]),
            "binl": np.ascontiguousarray(binl[sl]),
            "inst": np.ascontiguousarray(inst[sl]),
        })

    r = bass_utils.run_bass_kernel_spmd(nc, in_maps,
                                        core_ids=list(range(NCORES)))
    LAST_EXEC_TIME_NS = r.exec_time_ns

    S = np.stack([m["out"] for m in r.results]).reshape(
        NCORES * BPC, NLAB, G, NCH, G)
    Sd = np.einsum('bljcj->blc', S).astype(np.float64)        # [16,5,17]
    sums = Sd[:, :, 0:8]
    sumsq = Sd[:, :, 8:16].sum(-1)
    CT = np.stack([m["cnt"] for m in r.results]).reshape(
        NCORES * BPC, P, NT, NLAB)
    counts = CT.astype(np.float64).sum(axis=(1, 2))           # [16,5]

    loss = _loss_from_stats(sums, sumsq, counts)
    return np.array(loss, dtype=np.float32)


# revision 28
# speedup vs baseline: 75322.0541x; 1.0173x over previous
"""Discriminative-loss (clustering) kernel for Trainium2, 8 NeuronCores.

Strategy: pure data parallelism over the batch (B=16 -> 2 images/core).
Per image, the heavy work is a segmented (per-label) reduction over
524288 pixels:
    sums[l, e]  = sum_p  mask_l(p) * binary(p) * pred[e, p]
    sumsq[l, e] = sum_p  mask_l(p) * binary(p) * pred[e, p]^2
    counts[l]   = sum_p  mask_l(p)
computed on-device; the tiny remaining math (means, hinge terms,
pairwise distances -> scalar loss) is done on the host in float64.

Device mapping per image (pixels viewed as [128 partitions, 4096 cols]):
  - DMA: pred/binary are cast fp32->bf16 on the way in (SWDGE). Random
    rounding errors cancel in the ~1e5-element sums, so the loss keeps
    ~1e-4 accuracy while halving SBUF and letting the PE run at full
    bf16 rate with no moving-size constraints.
  - VectorE: keys t = inst+1 and s = t*binary in bf16, then per label
    bmask_l = [s==l+1] (the matmul weights) and [t==l+1] whose per-
    partition reduction (accum_out) yields the raw pixel counts; the
    is_equal ops run at the 4x DVE mode.
  - ScalarE: pred^2 for all 8 channels in one Square activation over a
    [128, 8, F] view.
  - TensorE: block-diagonal grouped matmul. Each matmul contracts 128
    pixels/column for G=24 columns at once:
      lhsT = bmasks [128, (5 labels, G)]  bf16  -> M = 120 (one
             contiguous group-interleaved block; walrus wants a single
             free dim on the weights AP)
      rhs  = data   [128, (16 chans, G)]  bf16  -> N = 384
    accumulated over all 171 groups of an image into one [120, 384] PSUM
    region. Useful entries are psum[lab*G+j, chan*G+j].
"""

import numpy as np

import concourse.mybir as mybir
from concourse import bacc, bass_utils
from concourse.tile import TileContext

P = 128            # SBUF partitions
F = 360            # pixel columns per pipeline tile (multiple of G); smallish
                   # tiles keep TensorE bursts dense (HAM stays warm) and
                   # deepen the DMA/compute pipeline
G = 24             # pixel-chunk columns per matmul group
NLAB = 5
NCH = 16           # rhs slots: 0-7 pred, 8-15 pred^2
M = NLAB * G       # 120 psum partitions
N = NCH * G        # 384 psum columns
BPC = 2            # images per core
NCORES = 8
DELTA_V = 0.5
DELTA_D = 3.0

# (col0, real_cols, padded_cols) per pipeline tile; 11*360 + 136 = 4096.
TILES = [(i * F, F, F) for i in range(11)] + [(11 * F, 136, 144)]
NT = len(TILES)
NGROUPS = sum(cp // G for _, _, cp in TILES)  # 171

LAST_EXEC_TIME_NS = None

_nc_cache = []


def _build():
    f32, bf16, i32 = mybir.dt.float32, mybir.dt.bfloat16, mybir.dt.int32
    op = mybir.AluOpType

    nc = bacc.Bacc("TRN2", target_bir_lowering=False, num_swdge_queues=4)
    pred = nc.dram_tensor("pred", [BPC, 8, 512, 1024], f32, kind="ExternalInput")
    binl = nc.dram_tensor("binl", [BPC, 512, 1024], f32, kind="ExternalInput")
    inst = nc.dram_tensor("inst", [BPC, 512, 1024], i32, kind="ExternalInput")
    out = nc.dram_tensor("out", [BPC, M, N], f32, kind="ExternalOutput")
    cnt = nc.dram_tensor("cnt", [BPC, P, NT * NLAB], f32, kind="ExternalOutput")

    pred_v = pred.rearrange("b e (p a) w -> b p e (a w)", p=P)  # [2,128,8,4096]
    bin_v = binl.rearrange("b (p a) w -> b p (a w)", p=P)       # [2,128,4096]
    inst_v = inst.rearrange("b (p a) w -> b p (a w)", p=P)

    with TileContext(nc) as tc:
        with tc.tile_pool(name="io", bufs=6) as io, \
             tc.tile_pool(name="wk", bufs=4) as wk, \
             tc.tile_pool(name="ps", bufs=2, space="PSUM") as ps, \
             tc.tile_pool(name="res", bufs=2) as res:
            for b in range(BPC):
                psum = ps.tile([M, N], f32, tag="psum")
                ct = res.tile([P, NT * NLAB], f32, tag="ct")
                k = 0
                for ti, (c0, creal, cpad) in enumerate(TILES):
                    rhs = io.tile([P, NCH, F], bf16, tag="rhs")
                    msk = wk.tile([P, F // G, NLAB * G], bf16, tag="msk")
                    it = io.tile([P, F], i32, tag="it")
                    btf = io.tile([P, F], f32, tag="btf")
                    sk = wk.tile([P, F], bf16, tag="sk")
                    tk = wk.tile([P, F], bf16, tag="tk")
                    junk = wk.tile([P, F], bf16, tag="junk")

                    # pred: fp32 -> bf16 cast inside the SWDGE DMA (Pool
                    # engine); inst/binary ride the cheaper HWDGE path and
                    # binary is cast on DVE.
                    nc.gpsimd.dma_start(out=rhs[:, 0:8, 0:creal],
                                        in_=pred_v[b, :, :, c0:c0 + creal])
                    nc.sync.dma_start(out=btf[:, 0:creal],
                                      in_=bin_v[b, :, c0:c0 + creal])
                    nc.sync.dma_start(out=it[:, 0:creal],
                                      in_=inst_v[b, :, c0:c0 + creal])
                    if cpad > creal:
                        # inst=-1 -> both keys miss every label; pred=0 keeps
                        # NaN garbage out of the accumulated products.
                        nc.vector.memset(it[:, creal:cpad], -1)
                        nc.vector.memset(btf[:, creal:cpad], 0.0)
                        nc.vector.memset(rhs[:, 0:8, creal:cpad], 0.0)

                    nc.vector.tensor_scalar(out=tk[:, 0:cpad], in0=it[:, 0:cpad],
                                            scalar1=1.0, scalar2=None, op0=op.add)
                    nc.vector.tensor_mul(out=sk[:, 0:cpad], in0=tk[:, 0:cpad],
                                         in1=btf[:, 0:cpad])
                    gmax = cpad // G
                    sk_v = sk[:, 0:cpad].rearrange("p (g j) -> p g j", j=G)
                    for lab in range(NLAB):
                        nc.vector.tensor_scalar(out=msk[:, 0:gmax,
                                                        lab * G:(lab + 1) * G],
                                                in0=sk_v,
                                                scalar1=float(lab + 1),
                                                scalar2=None, op0=op.is_equal)
                        # Raw-label compare: tensor discarded, per-partition
                        # sum (accum_out) is the label's pixel count.
                        nc.vector.tensor_scalar(out=junk[:, 0:cpad],
                                                in0=tk[:, 0:cpad],
                                                scalar1=float(lab + 1),
                                                scalar2=0.0, op0=op.is_equal,
                                                op1=op.add,
                                                accum_out=ct[:, ti * NLAB + lab:
                                                             ti * NLAB + lab + 1])
                    nc.scalar.activation(out=rhs[:, 8:16, 0:cpad],
                                         in_=rhs[:, 0:8, 0:cpad],
                                         func=mybir.ActivationFunctionType.Square)

                    for g in range(cpad // G):
                        j0 = g * G
                        nc.tensor.matmul(
                            psum[:, :],
                            msk[:, g, :],
                            rhs[:, :, j0:j0 + G],
                            start=(k == 0),
                            stop=(k == NGROUPS - 1),
                        )
                        k += 1
                ot = res.tile([M, N], f32, tag="ot")
                nc.vector.tensor_copy(out=ot[:, :], in_=psum[:, :])
                nc.gpsimd.dma_start(out=out[b], in_=ot[:, :])
                nc.gpsimd.dma_start(out=cnt[b], in_=ct[:, :])
    # bacc lowering: splits multi-wait sync_info into EventSemaphore
    # instructions (TRN2 allows 1 wait/instruction), inserts ACT table
    # loads, allocates registers.
    nc.compile()
    return nc


def _get_nc():
    if not _nc_cache:
        _nc_cache.append(_build())
    return _nc_cache[0]


def _loss_from_stats(sums, sumsq, counts):
    """Mirror of the reference loss math, in float64. Inputs are [B,5,8],
    [B,5], [B,5]."""
    C = NLAB - 1
    with np.errstate(divide="ignore", invalid="ignore"):
        mu = sums / counts[..., None]                         # [B,5,8]
    frob = sumsq - counts * np.sum(mu * mu, axis=-1)          # [B,5]
    pos = frob > 0
    n = np.where(pos, np.sqrt(np.where(pos, frob, 1.0)), 0.0)
    var = np.where(n > DELTA_V, (n - DELTA_V) ** 2, 0.0)
    l_var = np.sum(var, axis=1) / C                           # [B]

    mu_d = mu[:, :C]                                          # [B,4,8]
    diff = mu_d[:, :, None, :] - mu_d[:, None, :, :]
    dsq = np.sum(diff * diff, axis=-1)                        # [B,4,4]
    offdiag = (1.0 - np.eye(C))[None]
    ok = (dsq > 0) & (offdiag > 0)
    d = np.sqrt(np.where(ok, dsq, 1.0))
    hinge = np.where(ok, np.maximum(DELTA_D - d, 0.0) ** 2,
                     np.where(offdiag > 0, DELTA_D ** 2, 0.0))
    l_dist = np.sum(hinge, axis=(1, 2))                       # [B]
    return np.mean(l_var) + np.mean(l_dist)


def kernel(pred, binary_label, instance_label):
    global LAST_EXEC_TIME_NS
    pred = np.ascontiguousarray(pred, dtype=np.float32)
    binl = np.ascontiguousarray(binary_label, dtype=np.float32).reshape(
        pred.shape[0], 512, 1024)
    inst = np.ascontiguousarray(instance_label, dtype=np.int32)

    nc = _get_nc()
    in_maps = []
    for c in range(NCORES):
        sl = slice(BPC * c, BPC * (c + 1))
        in_maps.append({
            "pred": np.ascontiguousarray(pred[sl]),
            "binl": np.ascontiguousarray(binl[sl]),
            "inst": np.ascontiguousarray(inst[sl]),
        })

    r = bass_utils.run_bass_kernel_spmd(nc, in_maps,
                                        core_ids=list(range(NCORES)))
    LAST_EXEC_TIME_NS = r.exec_time_ns

    S = np.stack([m["out"] for m in r.results]).reshape(
        NCORES * BPC, NLAB, G, NCH, G).astype(np.float64)
    Sd = np.einsum('bljcj->blc', S)                           # [16,5,16]
    sums = Sd[:, :, 0:8]
    sumsq = Sd[:, :, 8:16].sum(-1)
    CT = np.stack([m["cnt"] for m in r.results]).reshape(
        NCORES * BPC, P, NT, NLAB)
    counts = CT.astype(np.float64).sum(axis=(1, 2))           # [16,5]

    loss = _loss_from_stats(sums, sumsq, counts)
    return np.array(loss, dtype=np.float32)


# revision 33
# speedup vs baseline: 77212.5314x; 1.0251x over previous
"""Discriminative-loss (clustering) kernel for Trainium2, 8 NeuronCores.

Strategy: pure data parallelism over the batch (B=16 -> 2 images/core).
Per image, the heavy work is a segmented (per-label) reduction over
524288 pixels:
    sums[l, e]  = sum_p  mask_l(p) * binary(p) * pred[e, p]
    sumsq[l, e] = sum_p  mask_l(p) * binary(p) * pred[e, p]^2
    counts[l]   = sum_p  mask_l(p)
computed on-device; the tiny remaining math (means, hinge terms,
pairwise distances -> scalar loss) is done on the host in float64.

Device mapping per image (pixels viewed as [128 partitions, 4096 cols]):
  - DMA: pred/binary are cast fp32->bf16 on the way in (SWDGE). Random
    rounding errors cancel in the ~1e5-element sums, so the loss keeps
    ~1e-4 accuracy while halving SBUF and letting the PE run at full
    bf16 rate with no moving-size constraints.
  - VectorE: keys t = inst+1 and s = t*binary in bf16, then per label
    bmask_l = [s==l+1] (the matmul weights) and [t==l+1] whose per-
    partition reduction (accum_out) yields the raw pixel counts; the
    is_equal ops run at the 4x DVE mode.
  - ScalarE: pred^2 for all 8 channels in one Square activation over a
    [128, 8, F] view.
  - TensorE: block-diagonal grouped matmul. Each matmul contracts 128
    pixels/column for G=24 columns at once:
      lhsT = bmasks [128, (5 labels, G)]  bf16  -> M = 120 (one
             contiguous group-interleaved block; walrus wants a single
             free dim on the weights AP)
      rhs  = data   [128, (16 chans, G)]  bf16  -> N = 384
    accumulated over all 171 groups of an image into one [120, 384] PSUM
    region. Useful entries are psum[lab*G+j, chan*G+j].
"""

import numpy as np

import concourse.mybir as mybir
from concourse import bacc, bass_utils
from concourse.tile import TileContext

P = 128            # SBUF partitions
F = 360            # pixel columns per pipeline tile (multiple of G); smallish
                   # tiles keep TensorE bursts dense (HAM stays warm) and
                   # deepen the DMA/compute pipeline
G = 24             # pixel-chunk columns per matmul group
NLAB = 5
NCH = 16           # rhs slots: 0-7 pred, 8-15 pred^2
M = NLAB * G       # 120 psum partitions
N = NCH * G        # 384 psum columns
BPC = 2            # images per core
NCORES = 8
DELTA_V = 0.5
DELTA_D = 3.0

# (col0, real_cols, padded_cols) per pipeline tile; 11*360 + 136 = 4096.
TILES = [(i * F, F, F) for i in range(11)] + [(11 * F, 136, 144)]
NT = len(TILES)
NGROUPS = sum(cp // G for _, _, cp in TILES)  # 171

LAST_EXEC_TIME_NS = None

_nc_cache = []


def _build():
    f32, bf16, i32 = mybir.dt.float32, mybir.dt.bfloat16, mybir.dt.int32
    op = mybir.AluOpType

    nc = bacc.Bacc("TRN2", target_bir_lowering=False, num_swdge_queues=4)
    pred = nc.dram_tensor("pred", [BPC, 8, 512, 1024], f32, kind="ExternalInput")
    binl = nc.dram_tensor("binl", [BPC, 512, 1024], f32, kind="ExternalInput")
    inst = nc.dram_tensor("inst", [BPC, 512, 1024], i32, kind="ExternalInput")
    # Single packed output per image: cols [0, N) rows [0, M) hold the psum
    # statistics, cols [N, N + NT*NLAB) hold the per-partition count partials.
    out = nc.dram_tensor("out", [BPC, P, N + NT * NLAB], f32,
                         kind="ExternalOutput")

    pred_v = pred.rearrange("b e (p a) w -> b p e (a w)", p=P)  # [2,128,8,4096]
    bin_v = binl.rearrange("b (p a) w -> b p (a w)", p=P)       # [2,128,4096]
    inst_v = inst.rearrange("b (p a) w -> b p (a w)", p=P)

    with TileContext(nc) as tc:
        with tc.tile_pool(name="io", bufs=6) as io, \
             tc.tile_pool(name="wk", bufs=4) as wk, \
             tc.tile_pool(name="ps", bufs=2, space="PSUM") as ps, \
             tc.tile_pool(name="res", bufs=2) as res:
            for b in range(BPC):
                psum = ps.tile([M, N], f32, tag="psum")
                ot = res.tile([P, N + NT * NLAB], f32, tag="ot")
                ct = ot[:, N:N + NT * NLAB]
                k = 0
                for ti, (c0, creal, cpad) in enumerate(TILES):
                    rhs = io.tile([P, NCH, F], bf16, tag="rhs")
                    msk = wk.tile([P, F // G, NLAB * G], bf16, tag="msk")
                    it = io.tile([P, F], i32, tag="it")
                    btf = io.tile([P, F], f32, tag="btf")
                    sk = wk.tile([P, F], bf16, tag="sk")
                    tk = wk.tile([P, F], bf16, tag="tk")
                    junk = wk.tile([P, F], bf16, tag="junk")

                    # pred: fp32 -> bf16 cast inside the SWDGE DMA (Pool
                    # engine); inst/binary ride the cheaper HWDGE path and
                    # binary is cast on DVE.
                    nc.gpsimd.dma_start(out=rhs[:, 0:8, 0:creal],
                                        in_=pred_v[b, :, :, c0:c0 + creal])
                    nc.sync.dma_start(out=btf[:, 0:creal],
                                      in_=bin_v[b, :, c0:c0 + creal])
                    nc.sync.dma_start(out=it[:, 0:creal],
                                      in_=inst_v[b, :, c0:c0 + creal])
                    if cpad > creal:
                        # inst=-1 -> both keys miss every label; pred=0 keeps
                        # NaN garbage out of the accumulated products.
                        nc.vector.memset(it[:, creal:cpad], -1)
                        nc.vector.memset(btf[:, creal:cpad], 0.0)
                        nc.vector.memset(rhs[:, 0:8, creal:cpad], 0.0)

                    nc.vector.tensor_scalar(out=tk[:, 0:cpad], in0=it[:, 0:cpad],
                                            scalar1=1.0, scalar2=None, op0=op.add)
                    nc.vector.tensor_mul(out=sk[:, 0:cpad], in0=tk[:, 0:cpad],
                                         in1=btf[:, 0:cpad])
                    gmax = cpad // G
                    sk_v = sk[:, 0:cpad].rearrange("p (g j) -> p g j", j=G)
                    for lab in range(NLAB):
                        nc.vector.tensor_scalar(out=msk[:, 0:gmax,
                                                        lab * G:(lab + 1) * G],
                                                in0=sk_v,
                                                scalar1=float(lab + 1),
                                                scalar2=None, op0=op.is_equal)
                        # Raw-label compare: tensor discarded, per-partition
                        # sum (accum_out) is the label's pixel count.
                        nc.vector.tensor_scalar(out=junk[:, 0:cpad],
                                                in0=tk[:, 0:cpad],
                                                scalar1=float(lab + 1),
                                                scalar2=0.0, op0=op.is_equal,
                                                op1=op.add,
                                                accum_out=ct[:, ti * NLAB + lab:
                                                             ti * NLAB + lab + 1])
                    nc.scalar.activation(out=rhs[:, 8:16, 0:cpad],
                                         in_=rhs[:, 0:8, 0:cpad],
                                         func=mybir.ActivationFunctionType.Square)

                    for g in range(cpad // G):
                        j0 = g * G
                        nc.tensor.matmul(
                            psum[:, :],
                            msk[:, g, :],
                            rhs[:, :, j0:j0 + G],
                            start=(k == 0),
                            stop=(k == NGROUPS - 1),
                        )
                        k += 1
                nc.vector.tensor_copy(out=ot[0:M, 0:N], in_=psum[:, :])
                # Rows M..P of the stats columns are never written; the host
                # slices them away. One HWDGE DMA drains stats + counts.
                nc.sync.dma_start(out=out[b], in_=ot[:, :])
    # bacc lowering: splits multi-wait sync_info into EventSemaphore
    # instructions (TRN2 allows 1 wait/instruction), inserts ACT table
    # loads, allocates registers.
    nc.compile()
    return nc


def _get_nc():
    if not _nc_cache:
        _nc_cache.append(_build())
    return _nc_cache[0]


def _loss_from_stats(sums, sumsq, counts):
    """Mirror of the reference loss math, in float64. Inputs are [B,5,8],
    [B,5], [B,5]."""
    C = NLAB - 1
    with np.errstate(divide="ignore", invalid="ignore"):
        mu = sums / counts[..., None]                         # [B,5,8]
    frob = sumsq - counts * np.sum(mu * mu, axis=-1)          # [B,5]
    pos = frob > 0
    n = np.where(pos, np.sqrt(np.where(pos, frob, 1.0)), 0.0)
    var = np.where(n > DELTA_V, (n - DELTA_V) ** 2, 0.0)
    l_var = np.sum(var, axis=1) / C                           # [B]

    mu_d = mu[:, :C]                                          # [B,4,8]
    diff = mu_d[:, :, None, :] - mu_d[:, None, :, :]
    dsq = np.sum(diff * diff, axis=-1)                        # [B,4,4]
    offdiag = (1.0 - np.eye(C))[None]
    ok = (dsq > 0) & (offdiag > 0)
    d = np.sqrt(np.where(ok, dsq, 1.0))
    hinge = np.where(ok, np.maximum(DELTA_D - d, 0.0) ** 2,
                     np.where(offdiag > 0, DELTA_D ** 2, 0.0))
    l_dist = np.sum(hinge, axis=(1, 2))                       # [B]
    return np.mean(l_var) + np.mean(l_dist)


def kernel(pred, binary_label, instance_label):
    global LAST_EXEC_TIME_NS
    pred = np.ascontiguousarray(pred, dtype=np.float32)
    binl = np.ascontiguousarray(binary_label, dtype=np.float32).reshape(
        pred.shape[0], 512, 1024)
    inst = np.ascontiguousarray(instance_label, dtype=np.int32)

    nc = _get_nc()
    in_maps = []
    for c in range(NCORES):
        sl = slice(BPC * c, BPC * (c + 1))
        in_maps.append({
            "pred": np.ascontiguousarray(pred[sl]),
            "binl": np.ascontiguousarray(binl[sl]),
            "inst": np.ascontiguousarray(inst[sl]),
        })

    r = bass_utils.run_bass_kernel_spmd(nc, in_maps,
                                        core_ids=list(range(NCORES)))
    LAST_EXEC_TIME_NS = r.exec_time_ns

    packed = np.stack([m["out"] for m in r.results]).reshape(
        NCORES * BPC, P, N + NT * NLAB).astype(np.float64)
    S = packed[:, 0:M, 0:N].reshape(NCORES * BPC, NLAB, G, NCH, G)
    Sd = np.einsum('bljcj->blc', S)                           # [16,5,16]
    sums = Sd[:, :, 0:8]
    sumsq = Sd[:, :, 8:16].sum(-1)
    CT = packed[:, :, N:].reshape(NCORES * BPC, P, NT, NLAB)
    counts = CT.sum(axis=(1, 2))                              # [16,5]

    loss = _loss_from_stats(sums, sumsq, counts)
    return np.array(loss, dtype=np.float32)
